# revision 6
# baseline (speedup 1.0000x reference)
"""Trainium2 Bass kernel for the pre-LN multi-head attention block.

Sharding: 8 cores = 4 batches x 2 query-row halves, collective-free. Each core
computes all 16 heads for its 512 query rows, with full-T k/v for its batch
(k/v compute duplicated across the 2 cores of a batch).

Per-core scheme (C=1024 channels, T=1024 rows, TQ=512 query rows):
  - everything is bf16 into the PE (1 cyc/row + fast weight load); PSUM
    accumulates fp32. Host pre-casts x^T and all weights to bf16 and lays the
    weights out slab-contiguous so each weight matrix is ONE [128, 8192] DMA
    with 16KB-per-partition contiguous rows (128 descriptors, full HBM BW).
  - LN stats via bf16 ones-matmuls, 4-way column-tiled (tile_position col
    groups) so the sum/sumsq matmuls for the two 512-col halves can overlap
    in the PE array; rstd = exp(-0.5*ln(var/n+eps)) on ACT (natural_log_exp
    table set shared with attention's exp; a dummy Ln at t=0 pre-loads it).
  - rowchain per 512-col half; mean/rstd rows broadcast across partitions
    with a K=1 ones-matmul on the PE (PSUM), then one DVE copy to bf16 SBUF;
    normalize = 2 bf16 DVE ops per chunk-half, emitted half-0-first so the
    q matmuls (which only need query columns 0:512) start early.
  - q^T / k^T keep channels on partitions; qk-LN gain/bias applied on ACT.
  - v [T, C] is bf16, head-interleaved with a ones column every 65 cols
    (softmax denominator accumulates as row 64 of the av psum). All v matmul
    groups run before attention pair emission so attention is exp/ACT-bound.
  - scores^T per head pair = 2 matmuls (K=64 halves of the chunk) which the
    PE runs concurrently via row-group tiling; exp on ACT (scale=0.125 folded
    in, no max-subtraction); p stored bf16, pool 2 pairs deep.
  - attn@v: both heads via 65-col augmented v (bf16), psum pool of 3 banks.
  - denominators: psum row 64 -> SBUF (DVE), reciprocal_approx_fast, GpSimd
    partition_broadcast to [64, 2*TQ], then the PSUM drain of av values fuses
    the 1/den scaling (DVE tensor_tensor, banks disjoint from ACT's).
  - proj: y^T = Wp^T out^T + bias; double-buffered psum; host transposes.
"""

from contextlib import ExitStack

import ml_dtypes
import numpy as np

import concourse.bacc as bacc
import concourse.mybir as mybir
import concourse.tile as tile
from concourse.bass_utils import run_bass_kernel_spmd

F32 = mybir.dt.float32
BF16 = mybir.dt.bfloat16
AF = mybir.ActivationFunctionType
OP = mybir.AluOpType

B, T, C = 4, 1024, 1024
H, D = 16, 64
TQ = 512           # query rows per core
NCH = 8            # 128-row chunks of C (or T)
EPS = 1e-5

_CACHE = {}


def _build():
    nc = bacc.Bacc(None, target_bir_lowering=False, debug=False)

    xT_d = nc.declare_dram_parameter("xT", [C, T], BF16, isOutput=False)
    # weights pre-laid by host: [128, 8192] slab-contiguous (see _prep_inputs)
    wq_d = nc.declare_dram_parameter("wq", [128, NCH * C], BF16, isOutput=False)
    wk_d = nc.declare_dram_parameter("wk", [128, NCH * C], BF16, isOutput=False)
    wv_d = nc.declare_dram_parameter("wv", [128, NCH * C], BF16, isOutput=False)
    wp_d = nc.declare_dram_parameter("wp", [128, NCH * C], BF16, isOutput=False)
    bq_d = nc.declare_dram_parameter("bq", [C], F32, isOutput=False)
    bk_d = nc.declare_dram_parameter("bk", [C], F32, isOutput=False)
    bv_d = nc.declare_dram_parameter("bv", [C], F32, isOutput=False)
    bp_d = nc.declare_dram_parameter("bp", [C], F32, isOutput=False)
    qg_d = nc.declare_dram_parameter("qg", [C], F32, isOutput=False)
    qb_d = nc.declare_dram_parameter("qb", [C], F32, isOutput=False)
    kg_d = nc.declare_dram_parameter("kg", [C], F32, isOutput=False)
    kb_d = nc.declare_dram_parameter("kb", [C], F32, isOutput=False)
    yT_d = nc.declare_dram_parameter("yT", [C, TQ], F32, isOutput=True)

    with tile.TileContext(nc) as tc, ExitStack() as ctx:
        pool = tc.tile_pool

        const = ctx.enter_context(pool(name="const", bufs=1))
        wqp = ctx.enter_context(pool(name="wqp", bufs=1))
        wkp = ctx.enter_context(pool(name="wkp", bufs=1))
        wvp = ctx.enter_context(pool(name="wvp", bufs=1))
        wpp = ctx.enter_context(pool(name="wpp", bufs=1))
        qsbp = ctx.enter_context(pool(name="qsb", bufs=1))
        ksbp = ctx.enter_context(pool(name="ksb", bufs=1))
        vsbp = ctx.enter_context(pool(name="vsb", bufs=1))
        osbp = ctx.enter_context(pool(name="osb", bufs=1))

        # ============ big-load FIFO: x chunks, then all weights ============
        xz_ctx = ExitStack()
        xzp = xz_ctx.enter_context(pool(name="xz", bufs=1))
        xts = []
        for j in range(NCH):
            t = xzp.tile([128, T], BF16, tag=f"x{j}")
            nc.sync.dma_start(out=t, in_=xT_d[j * 128:(j + 1) * 128, :])
            xts.append(t)

        wq_sb = wqp.tile([128, NCH * C], BF16)
        nc.sync.dma_start(out=wq_sb, in_=wq_d.ap())
        wk_sb = wkp.tile([128, NCH * C], BF16)
        nc.sync.dma_start(out=wk_sb, in_=wk_d.ap())
        wv_sb = wvp.tile([128, NCH * C], BF16)
        nc.sync.dma_start(out=wv_sb, in_=wv_d.ap())
        wp_sb = wpp.tile([128, NCH * C], BF16)
        nc.sync.dma_start(out=wp_sb, in_=wp_d.ap())

        # matmul-facing views
        wq_v = wq_sb.rearrange("p (m j c) -> p m j c", m=NCH, j=NCH)
        wk_v = wk_sb.rearrange("p (m j c) -> p m j c", m=NCH, j=NCH)
        wv_v = wv_sb.rearrange("p (g j c) -> p g j c", g=4, j=NCH)
        wp_v = wp_sb.rearrange("p (j c) -> p j c", j=NCH)

        # small consts behind the big loads in the FIFO
        def vec8(name, d):
            t = const.tile([128, 8], F32, tag=name)
            nc.sync.dma_start(out=t, in_=d.ap().rearrange("(j p) -> p j", p=128))
            return t

        bq8 = vec8("bq8", bq_d)
        bk8 = vec8("bk8", bk_d)
        bp8 = vec8("bp8", bp_d)
        qg8 = vec8("qg8", qg_d)
        qb8 = vec8("qb8", qb_d)
        kg8 = vec8("kg8", kg_d)
        kb8 = vec8("kb8", kb_d)
        bvb = const.tile([128, C], F32)
        nc.sync.dma_start(out=bvb, in_=bv_d.ap().rearrange("c -> () c").to_broadcast([128, C]))

        # ---- constants ----
        ones_blk = const.tile([128, 128], F32, tag="onesblk")
        nc.vector.memset(ones_blk, 1.0)
        ones1b = const.tile([128, 1], BF16, tag="ones1b")
        nc.vector.tensor_copy(out=ones1b, in_=ones_blk[:, 0:1])
        ones_row = const.tile([1, 128], BF16, tag="onesrow")
        nc.vector.tensor_copy(out=ones_row, in_=ones_blk[0:1, :])
        eps1 = const.tile([1, 1], F32)
        nc.vector.memset(eps1, EPS)
        scr1 = const.tile([1, 1], F32, tag="scr1")
        # dummy Ln at t=0: pre-loads the natural_log_exp ACT table set so the
        # first real Ln/Exp isn't stalled behind a ~2.7us ACT_TABLE_LOAD
        nc.scalar.activation(out=scr1, in_=eps1, func=AF.Ln, bias=eps1, scale=1.0)

        # persistent activations
        q_sb = qsbp.tile([128, NCH, TQ], BF16)      # q^T, later q-hat
        k_sb = ksbp.tile([128, NCH, T], BF16)       # k^T, later k-hat
        v_sb = vsbp.tile([128, NCH, H * 65], BF16)  # v head-interleaved + ones col
        outT_sb = osbp.tile([128, NCH, TQ], BF16)

        # temp pools for the LN phases (closed before attention pools open)
        tmp_ctx = ExitStack()
        rows = tmp_ctx.enter_context(pool(name="rows", bufs=2))
        packp = tmp_ctx.enter_context(pool(name="pack", bufs=1))
        mrp = tmp_ctx.enter_context(pool(name="mr", bufs=1))
        sqp = tmp_ctx.enter_context(pool(name="sq", bufs=2))

        def rowchain(sum_ap, sq_ap, n, pack):
            """pack[:, 0:n] = mean, pack[:, n:2n] = rstd (bf16) from raw
            column-sum / column-sumsq rows (PSUM)."""
            mu32 = rows.tile([1, 512], F32, tag="rmu")
            nc.vector.tensor_scalar(out=mu32[:, 0:n], in0=sum_ap, scalar1=1.0 / C,
                                    scalar2=None, op0=OP.mult)
            t2 = rows.tile([1, 512], F32, tag="rt2")
            nc.vector.tensor_tensor(out=t2[:, 0:n], in0=sum_ap, in1=mu32[:, 0:n], op=OP.mult)
            d = rows.tile([1, 512], F32, tag="rd")
            nc.vector.tensor_tensor(out=d[:, 0:n], in0=sq_ap, in1=t2[:, 0:n], op=OP.subtract)
            nc.vector.tensor_copy(out=pack[:, 0:n], in_=mu32[:, 0:n])
            nc.scalar.activation(out=d[:, 0:n], in_=d[:, 0:n], func=AF.Ln,
                                 bias=eps1, scale=1.0 / C)
            nc.scalar.activation(out=pack[:, n:2 * n], in_=d[:, 0:n], func=AF.Exp, scale=-0.5)

        def bc_rows(pack, n, bc_ps):
            """broadcast pack [1, 2n] across partitions via K=1 ones-matmul."""
            for s in range(0, 2 * n, 512):
                e = min(s + 512, 2 * n)
                nc.tensor.matmul(bc_ps[:, s:e], ones_row, pack[:, s:e],
                                 start=True, stop=True)

        # ================= phase A: x stats, normalize =================
        psA_ctx = ExitStack()
        psA = psA_ctx.enter_context(pool(name="psA", bufs=1, space="PSUM"))
        xstat_ps = psA.tile([97, 512], F32)
        for j in range(NCH):
            sqt = sqp.tile([128, T], BF16, tag="sqb")
            nc.vector.tensor_tensor(out=sqt, in0=xts[j], in1=xts[j], op=OP.mult)
            st, sp = j == 0, j == NCH - 1
            nc.tensor.matmul(xstat_ps[0:1, :], ones1b, xts[j][:, 0:512],
                             start=st, stop=sp, tile_position=(0, 0))
            nc.tensor.matmul(xstat_ps[32:33, :], ones1b, xts[j][:, 512:1024],
                             start=st, stop=sp, tile_position=(0, 32))
            nc.tensor.matmul(xstat_ps[64:65, :], ones1b, sqt[:, 0:512],
                             start=st, stop=sp, tile_position=(0, 64))
            nc.tensor.matmul(xstat_ps[96:97, :], ones1b, sqt[:, 512:1024],
                             start=st, stop=sp, tile_position=(0, 96))

        packx0 = packp.tile([1, 1024], BF16, tag="px0")
        packx1 = packp.tile([1, 1024], BF16, tag="px1")
        rowchain(xstat_ps[0:1, :], xstat_ps[64:65, :], 512, packx0)
        rowchain(xstat_ps[32:33, :], xstat_ps[96:97, :], 512, packx1)

        warm_ctx = ExitStack()
        warmp = warm_ctx.enter_context(pool(name="warm", bufs=1, space="PSUM"))
        warm_ps = warmp.tile([128, 512], F32)
        bcx_ctx = ExitStack()
        bcxp = bcx_ctx.enter_context(pool(name="bcx", bufs=2, space="PSUM"))
        # warm matmul keyed on packx0's mean (written ~0.7us into the chain):
        # keeps the PE HAM activity window hot through the rowchain stall
        nc.tensor.matmul(warm_ps, ones_row, packx0[:, 0:512], start=True, stop=True)

        mrx = mrp.tile([128, 4 * T], BF16)  # mu0|rs0|mu1|rs1 halves, bf16
        bcx0 = bcxp.tile([128, 1024], F32, tag="bch")
        bc_rows(packx0, 512, bcx0)
        nc.vector.tensor_copy(out=mrx[:, 0:1024], in_=bcx0)
        bcx1 = bcxp.tile([128, 1024], F32, tag="bch")
        bc_rows(packx1, 512, bcx1)
        nc.vector.tensor_copy(out=mrx[:, 1024:2048], in_=bcx1)

        # normalize in place, half 0 (query cols) first so q matmuls start asap
        for h in range(2):
            mu_ap = mrx[:, h * 1024:h * 1024 + 512]
            rs_ap = mrx[:, h * 1024 + 512:h * 1024 + 1024]
            for j in range(NCH):
                tz = sqp.tile([128, T], BF16, tag="sqb")
                nc.vector.tensor_tensor(out=tz[:, 0:512], in0=xts[j][:, h * 512:(h + 1) * 512],
                                        in1=mu_ap, op=OP.subtract)
                nc.vector.tensor_tensor(out=xts[j][:, h * 512:(h + 1) * 512],
                                        in0=tz[:, 0:512], in1=rs_ap, op=OP.mult)

        # ones columns of v (DVE, early; cols disjoint from v drains)
        v_ones_view = v_sb.rearrange("p i (h x) -> p i h x", x=65)[:, :, :, 64:65]
        nc.vector.tensor_copy(out=v_ones_view,
                              in_=ones_blk.rearrange("p (i h x) -> p i h x", i=NCH, h=H))

        # ================= phase B: q / k matmuls + their LNs =================
        qmm_ctx = ExitStack()
        qmmp = qmm_ctx.enter_context(pool(name="qmm", bufs=2, space="PSUM"))
        for m in range(NCH):
            q_ps = qmmp.tile([128, TQ], F32, tag="mm")
            for j in range(NCH):
                nc.tensor.matmul(q_ps, wq_v[:, m, j, :], xts[j][:, 0:TQ],
                                 start=(j == 0), stop=(j == NCH - 1))
            nc.scalar.activation(out=q_sb[:, m, :], in_=q_ps, func=AF.Identity,
                                 bias=bq8[:, m:m + 1], scale=1.0)
        qmm_ctx.close()
        bcx_ctx.close()
        warm_ctx.close()
        psA_ctx.close()

        # q stats (2-way col-tiled) + rowchain + broadcast + apply
        qs_ctx = ExitStack()
        qstatp = qs_ctx.enter_context(pool(name="qstat", bufs=1, space="PSUM"))
        qstat_ps = qstatp.tile([33, TQ], F32)
        for m in range(NCH):
            sqt = sqp.tile([128, T], BF16, tag="sqb")
            nc.vector.tensor_tensor(out=sqt[:, 0:TQ], in0=q_sb[:, m, :], in1=q_sb[:, m, :],
                                    op=OP.mult)
            st, sp = m == 0, m == NCH - 1
            nc.tensor.matmul(qstat_ps[0:1, :], ones1b, q_sb[:, m, :],
                             start=st, stop=sp, tile_position=(0, 0))
            nc.tensor.matmul(qstat_ps[32:33, :], ones1b, sqt[:, 0:TQ],
                             start=st, stop=sp, tile_position=(0, 32))

        # k matmuls (PE queue: ahead of the q rowchain's bc matmuls is fine --
        # the q-hat apply is hidden under these 128 matmuls)
        kmm_ctx = ExitStack()
        kmmp = kmm_ctx.enter_context(pool(name="kmm", bufs=2, space="PSUM"))

        packq = packp.tile([1, 1024], BF16, tag="pq")
        rowchain(qstat_ps[0:1, :], qstat_ps[32:33, :], TQ, packq)

        bcq_ctx = ExitStack()
        bcqp = bcq_ctx.enter_context(pool(name="bcq", bufs=1, space="PSUM"))
        mrq = mrp.tile([128, 2 * T], BF16, tag="mrq")

        k_drains = []
        for m in range(NCH):
            k_ps = kmmp.tile([128, T], F32, tag="mm")
            for n in range(2):
                for j in range(NCH):
                    nc.tensor.matmul(k_ps[:, n * 512:(n + 1) * 512], wk_v[:, m, j, :],
                                     xts[j][:, n * 512:(n + 1) * 512],
                                     start=(j == 0), stop=(j == NCH - 1))
            nc.scalar.activation(out=k_sb[:, m, :], in_=k_ps, func=AF.Identity,
                                 bias=bk8[:, m:m + 1], scale=1.0)
            if m == 0:
                # q broadcast matmuls slot in right after k[0]'s matmuls so the
                # q-hat applies (ACT/DVE) overlap the remaining k matmuls
                bcq0 = bcqp.tile([128, 2 * TQ], F32, tag="bq")
                bc_rows(packq, TQ, bcq0)
                nc.vector.tensor_copy(out=mrq[:, 0:2 * TQ], in_=bcq0)
                for mm_ in range(NCH):
                    t1 = sqp.tile([128, T], BF16, tag="sqb")
                    nc.vector.tensor_tensor(out=t1[:, 0:TQ], in0=q_sb[:, mm_, :],
                                            in1=mrq[:, 0:TQ], op=OP.subtract)
                    t2 = sqp.tile([128, T], BF16, tag="sqb")
                    nc.vector.tensor_tensor(out=t2[:, 0:TQ], in0=t1[:, 0:TQ],
                                            in1=mrq[:, TQ:2 * TQ], op=OP.mult)
                    nc.scalar.activation(out=q_sb[:, mm_, :], in_=t2[:, 0:TQ],
                                         func=AF.Identity, bias=qb8[:, mm_:mm_ + 1],
                                         scale=qg8[:, mm_:mm_ + 1])

        bcq_ctx.close()
        kmm_ctx.close()
        qs_ctx.close()

        # k stats (4-way col-tiled)
        ks_ctx = ExitStack()
        kstatp = ks_ctx.enter_context(pool(name="kstat", bufs=1, space="PSUM"))
        kstat_ps = kstatp.tile([97, 512], F32)
        for m in range(NCH):
            sqt = sqp.tile([128, T], BF16, tag="sqb")
            nc.vector.tensor_tensor(out=sqt, in0=k_sb[:, m, :], in1=k_sb[:, m, :], op=OP.mult)
            st, sp = m == 0, m == NCH - 1
            nc.tensor.matmul(kstat_ps[0:1, :], ones1b, k_sb[:, m, 0:512],
                             start=st, stop=sp, tile_position=(0, 0))
            nc.tensor.matmul(kstat_ps[32:33, :], ones1b, k_sb[:, m, 512:1024],
                             start=st, stop=sp, tile_position=(0, 32))
            nc.tensor.matmul(kstat_ps[64:65, :], ones1b, sqt[:, 0:512],
                             start=st, stop=sp, tile_position=(0, 64))
            nc.tensor.matmul(kstat_ps[96:97, :], ones1b, sqt[:, 512:1024],
                             start=st, stop=sp, tile_position=(0, 96))

        # ================= phase C: v matmuls, then attention =================
        vps_ctx = ExitStack()
        vpsp = vps_ctx.enter_context(pool(name="vps", bufs=1, space="PSUM"))

        packk0 = packp.tile([1, 1024], BF16, tag="pk0")
        packk1 = packp.tile([1, 1024], BF16, tag="pk1")
        rowchain(kstat_ps[0:1, :], kstat_ps[64:65, :], 512, packk0)
        rowchain(kstat_ps[32:33, :], kstat_ps[96:97, :], 512, packk1)

        bck_ctx = ExitStack()
        bckp = bck_ctx.enter_context(pool(name="bck", bufs=1, space="PSUM"))
        mrk = mrp.tile([128, 4 * T], BF16, tag="mrk")

        def vgroup(g):
            wvsl = wv_v[:, g]
            for i in range(NCH):
                v_ps = vpsp.tile([128, 256], F32, tag="vps")
                for j in range(NCH):
                    nc.tensor.matmul(v_ps, xts[j][:, i * 128:(i + 1) * 128],
                                     wvsl[:, j, :], start=(j == 0), stop=(j == NCH - 1))
                vout = v_sb.rearrange("p i (h x) -> p i h x", x=65)[:, i, g * 4:(g + 1) * 4, 0:64]
                vin = v_ps.rearrange("p (h x) -> p h x", x=64)
                nc.vector.tensor_tensor(
                    out=vout, in0=vin,
                    in1=bvb[:, g * 256:(g + 1) * 256].rearrange("p (h x) -> p h x", x=64),
                    op=OP.add)

        vgroup(0)
        # k broadcast + k-hat apply, hidden under the v matmuls
        bck0 = bckp.tile([128, 1024], F32, tag="bk")
        bc_rows(packk0, 512, bck0)
        nc.vector.tensor_copy(out=mrk[:, 0:1024], in_=bck0)
        vgroup(1)
        bck1 = bckp.tile([128, 1024], F32, tag="bk")
        bc_rows(packk1, 512, bck1)
        nc.vector.tensor_copy(out=mrk[:, 1024:2048], in_=bck1)

        for m in range(NCH):
            t1 = sqp.tile([128, T], BF16, tag="sqb")
            for h in range(2):
                nc.vector.tensor_tensor(out=t1[:, h * 512:(h + 1) * 512],
                                        in0=k_sb[:, m, h * 512:(h + 1) * 512],
                                        in1=mrk[:, h * 1024:h * 1024 + 512], op=OP.subtract)
            t2 = sqp.tile([128, T], BF16, tag="sqb")
            for h in range(2):
                nc.vector.tensor_tensor(out=t2[:, h * 512:(h + 1) * 512],
                                        in0=t1[:, h * 512:(h + 1) * 512],
                                        in1=mrk[:, h * 1024 + 512:h * 1024 + 1024], op=OP.mult)
            nc.scalar.activation(out=k_sb[:, m, :], in_=t2,
                                 func=AF.Identity, bias=kb8[:, m:m + 1],
                                 scale=kg8[:, m:m + 1])

        vgroup(2)
        vgroup(3)

        bck_ctx.close()
        vps_ctx.close()
        ks_ctx.close()
        tmp_ctx.close()
        xz_ctx.close()

        # ---- attention pairs ----
        att_ctx = ExitStack()
        pexpp = att_ctx.enter_context(pool(name="pexp", bufs=16))
        denp = att_ctx.enter_context(pool(name="den", bufs=1))
        rcbp = att_ctx.enter_context(pool(name="rcb", bufs=2))
        scp = att_ctx.enter_context(pool(name="sc", bufs=2, space="PSUM"))
        avp = att_ctx.enter_context(pool(name="av", bufs=3, space="PSUM"))

        p_tiles = {}

        def emit_scores(m):
            p_list = []
            for i in range(NCH):
                sc_ps = scp.tile([128, 1024], F32, tag="sc")
                nc.tensor.matmul(sc_ps[:, 0:512], k_sb[0:64, m, i * 128:(i + 1) * 128],
                                 q_sb[0:64, m, :], start=True, stop=True)
                nc.tensor.matmul(sc_ps[:, 512:1024], k_sb[64:128, m, i * 128:(i + 1) * 128],
                                 q_sb[64:128, m, :], start=True, stop=True)
                p_sb = pexpp.tile([128, 1024], BF16, tag="p")
                nc.scalar.activation(out=p_sb, in_=sc_ps[:, 0:1024], func=AF.Exp, scale=0.125)
                p_list.append(p_sb)
            p_tiles[m] = p_list

        def emit_av(m):
            p_list = p_tiles.pop(m)
            h0, h1 = 2 * m, 2 * m + 1
            av0 = avp.tile([65, TQ], F32, tag="av")
            av1 = avp.tile([65, TQ], F32, tag="av")
            for i in range(NCH):
                st, sp = i == 0, i == NCH - 1
                nc.tensor.matmul(av0, v_sb[:, i, h0 * 65:h0 * 65 + 65],
                                 p_list[i][:, 0:512], start=st, stop=sp)
                nc.tensor.matmul(av1, v_sb[:, i, h1 * 65:h1 * 65 + 65],
                                 p_list[i][:, 512:1024], start=st, stop=sp)
            dd = denp.tile([1, 2 * TQ], F32, tag="den")
            nc.vector.tensor_copy(out=dd[:, 0:TQ], in_=av0[64:65, :])
            nc.vector.tensor_copy(out=dd[:, TQ:2 * TQ], in_=av1[64:65, :])
            rt = denp.tile([1, 2 * TQ], F32, tag="rect")
            nc.vector.reciprocal_approx_fast(out=rt, in_=dd)
            rbb = rcbp.tile([64, 2 * TQ], F32, tag="rbb")
            nc.gpsimd.partition_broadcast(rbb, rt)
            nc.vector.tensor_tensor(out=outT_sb[0:64, m, :],
                                    in0=av0[0:64, :], in1=rbb[:, 0:TQ], op=OP.mult)
            nc.vector.tensor_tensor(out=outT_sb[64:128, m, :],
                                    in0=av1[0:64, :], in1=rbb[:, TQ:2 * TQ], op=OP.mult)

        for g in range(4):
            emit_scores(2 * g)
            emit_scores(2 * g + 1)
            emit_av(2 * g)
            emit_av(2 * g + 1)

        att_ctx.close()

        # ================= phase D: proj =================
        youtp = ctx.enter_context(pool(name="yout", bufs=2))
        pjp = ctx.enter_context(pool(name="pj", bufs=2, space="PSUM"))
        for m in range(NCH):
            y_ps = pjp.tile([128, TQ], F32, tag="pj")
            for j in range(NCH):
                nc.tensor.matmul(y_ps, wp_v[:, j, m * 128:(m + 1) * 128], outT_sb[:, j, :],
                                 start=(j == 0), stop=(j == NCH - 1))
            y_sb = youtp.tile([128, TQ], F32, tag="y")
            nc.scalar.activation(out=y_sb, in_=y_ps, func=AF.Identity,
                                 bias=bp8[:, m:m + 1], scale=1.0)
            nc.sync.dma_start(out=yT_d[m * 128:(m + 1) * 128, :], in_=y_sb)

    nc.finalize()
    return nc


def _get_nc():
    if "nc" not in _CACHE:
        _CACHE["nc"] = _build()
    return _CACHE["nc"]


def _lay_w(w, gcols):
    """[C, C] -> [128, 8192]: A[p, m*g*NCH*? ...] = w[j*128+p, m*gcols+c'],
    slab-major so each slab is contiguous per partition."""
    A = w.reshape(NCH, 128, C // gcols, gcols).transpose(1, 2, 0, 3)
    return np.ascontiguousarray(A.reshape(128, NCH * C))


def _prep_inputs(x, norm_g, norm_b, qkv_w, qkv_b, qln_g, qln_b, kln_g, kln_b, proj_w, proj_b):
    x = np.asarray(x, dtype=np.float32)
    norm_g = np.asarray(norm_g, dtype=np.float32)
    norm_b = np.asarray(norm_b, dtype=np.float32)
    qkv_w = np.asarray(qkv_w, dtype=np.float32)
    qkv_b = np.asarray(qkv_b, dtype=np.float32)

    wfold = norm_g[:, None] * qkv_w                    # [C, 3C]
    bfold = qkv_b + norm_b @ qkv_w                     # [3C]
    wq = np.ascontiguousarray(wfold[:, 0:C])
    wk = np.ascontiguousarray(wfold[:, C:2 * C])
    wv = np.ascontiguousarray(wfold[:, 2 * C:3 * C])
    bq, bk, bv = bfold[0:C].copy(), bfold[C:2 * C].copy(), bfold[2 * C:3 * C].copy()

    bf16 = ml_dtypes.bfloat16
    wp = np.ascontiguousarray(np.asarray(proj_w, dtype=np.float32))
    wp_lay = np.ascontiguousarray(wp.reshape(NCH, 128, C).transpose(1, 0, 2).reshape(128, NCH * C))
    common = dict(
        wq=_lay_w(wq, 128).astype(bf16), wk=_lay_w(wk, 128).astype(bf16),
        wv=_lay_w(wv, 256).astype(bf16), wp=wp_lay.astype(bf16),
        bq=bq, bk=bk, bv=bv,
        bp=np.asarray(proj_b, dtype=np.float32).copy(),
        qg=np.asarray(qln_g, dtype=np.float32).copy(),
        qb=np.asarray(qln_b, dtype=np.float32).copy(),
        kg=np.asarray(kln_g, dtype=np.float32).copy(),
        kb=np.asarray(kln_b, dtype=np.float32).copy(),
    )
    in_maps = []
    for core in range(8):
        b, half = core // 2, core % 2
        xp = np.concatenate([x[b, TQ * half:], x[b, :TQ * half]], axis=0) if half else x[b]
        xT = np.ascontiguousarray(xp.T).astype(bf16)
        in_maps.append(dict(common, xT=xT))
    return in_maps


def kernel(**inputs) -> np.ndarray:
    in_maps = _prep_inputs(**inputs)
    nc = _get_nc()
    res = run_bass_kernel_spmd(nc, in_maps, core_ids=list(range(8)))
    out = np.empty((B, T, C), dtype=np.float32)
    for core in range(8):
        b, half = core // 2, core % 2
        out[b, TQ * half:TQ * half + TQ, :] = res.results[core]["yT"].T
    return out


# revision 8
# speedup vs baseline: 1.0225x; 1.0225x over previous
"""Trainium2 Bass kernel for the pre-LN multi-head attention block.

Sharding: 8 cores = 4 batches x 2 query-row halves, collective-free. Each core
computes all 16 heads for its 512 query rows, with full-T k/v for its batch
(k/v compute duplicated across the 2 cores of a batch).

Per-core scheme (C=1024 channels, T=1024 rows, TQ=512 query rows):
  - everything is bf16 into the PE; PSUM accumulates fp32. Host pre-casts x^T
    and all weights to bf16 and lays the weights out slab-contiguous so each
    weight matrix is ONE [128, 8192] DMA (16KB contiguous per partition).
  - LN stats via bf16 ones-matmuls, column-tiled per 512-col half; the
    mean/rstd rowchain runs on 512-wide rows, with ONE batched Ln and ONE Exp
    per LN phase (minimizes ACT table-set switches); rows are broadcast
    across partitions with K=1 ones-matmuls on the PE then one DVE copy to
    bf16 SBUF; normalize = 2 bf16 DVE ops per chunk-half, half 0 first.
  - q matmuls run j-outer in two 4-output-chunk passes (4 PSUM banks each)
    so they overlap the tail of the x normalize.
  - v bias is folded into the proj bias on the host (bp' = bp + bv @ Wp), so
    v PSUM drains are plain copies; v psum is double-buffered. All v matmuls
    run before attention so attention is exp/ACT-bound.
  - scores^T per head pair = 2 matmuls (K=64 halves) which the PE runs
    concurrently via row-group tiling; exp on ACT over 2-chunk [128, 2048]
    groups (scale=0.125 folded in); p stored bf16.
  - attention is software-pipelined: score/exp groups of pair m interleave
    with the attn@v matmuls of pair m-1, so the PE has av work while exps
    pace the pipeline.
  - attn@v: both heads via 65-col augmented v (ones col -> denominator row);
    denominators: psum row 64 -> SBUF, reciprocal_approx_fast, GpSimd
    partition_broadcast, then the av PSUM drain fuses the 1/den scaling.
  - proj: y^T = Wp^T out^T + bias'; double-buffered psum; host transposes.
"""

from contextlib import ExitStack

import ml_dtypes
import numpy as np

import concourse.bacc as bacc
import concourse.mybir as mybir
import concourse.tile as tile
from concourse.bass_utils import run_bass_kernel_spmd

F32 = mybir.dt.float32
BF16 = mybir.dt.bfloat16
AF = mybir.ActivationFunctionType
OP = mybir.AluOpType

B, T, C = 4, 1024, 1024
H, D = 16, 64
TQ = 512           # query rows per core
NCH = 8            # 128-row chunks of C (or T)
EPS = 1e-5

_CACHE = {}


def _build():
    nc = bacc.Bacc(None, target_bir_lowering=False, debug=False)

    xT_d = nc.declare_dram_parameter("xT", [C, T], BF16, isOutput=False)
    wq_d = nc.declare_dram_parameter("wq", [128, NCH * C], BF16, isOutput=False)
    wk_d = nc.declare_dram_parameter("wk", [128, NCH * C], BF16, isOutput=False)
    wv_d = nc.declare_dram_parameter("wv", [128, NCH * C], BF16, isOutput=False)
    wp_d = nc.declare_dram_parameter("wp", [128, NCH * C], BF16, isOutput=False)
    bq_d = nc.declare_dram_parameter("bq", [C], F32, isOutput=False)
    bk_d = nc.declare_dram_parameter("bk", [C], F32, isOutput=False)
    bp_d = nc.declare_dram_parameter("bp", [C], F32, isOutput=False)
    qg_d = nc.declare_dram_parameter("qg", [C], F32, isOutput=False)
    qb_d = nc.declare_dram_parameter("qb", [C], F32, isOutput=False)
    kg_d = nc.declare_dram_parameter("kg", [C], F32, isOutput=False)
    kb_d = nc.declare_dram_parameter("kb", [C], F32, isOutput=False)
    yT_d = nc.declare_dram_parameter("yT", [C, TQ], F32, isOutput=True)

    with tile.TileContext(nc) as tc, ExitStack() as ctx:
        pool = tc.tile_pool

        const = ctx.enter_context(pool(name="const", bufs=1))
        wqp = ctx.enter_context(pool(name="wqp", bufs=1))
        wkp = ctx.enter_context(pool(name="wkp", bufs=1))
        wvp = ctx.enter_context(pool(name="wvp", bufs=1))
        wpp = ctx.enter_context(pool(name="wpp", bufs=1))
        qsbp = ctx.enter_context(pool(name="qsb", bufs=1))
        ksbp = ctx.enter_context(pool(name="ksb", bufs=1))
        vsbp = ctx.enter_context(pool(name="vsb", bufs=1))
        osbp = ctx.enter_context(pool(name="osb", bufs=1))

        # ============ big-load FIFO: x chunks, then all weights ============
        xz_ctx = ExitStack()
        xzp = xz_ctx.enter_context(pool(name="xz", bufs=1))
        xts = []
        for j in range(NCH):
            t = xzp.tile([128, T], BF16, tag=f"x{j}")
            nc.sync.dma_start(out=t, in_=xT_d[j * 128:(j + 1) * 128, :])
            xts.append(t)

        wq_sb = wqp.tile([128, NCH * C], BF16)
        nc.sync.dma_start(out=wq_sb, in_=wq_d.ap())
        wk_sb = wkp.tile([128, NCH * C], BF16)
        nc.sync.dma_start(out=wk_sb, in_=wk_d.ap())
        wv_sb = wvp.tile([128, NCH * C], BF16)
        nc.sync.dma_start(out=wv_sb, in_=wv_d.ap())
        wp_sb = wpp.tile([128, NCH * C], BF16)
        nc.sync.dma_start(out=wp_sb, in_=wp_d.ap())

        wq_v = wq_sb.rearrange("p (m j c) -> p m j c", m=NCH, j=NCH)
        wk_v = wk_sb.rearrange("p (m j c) -> p m j c", m=NCH, j=NCH)
        wv_v = wv_sb.rearrange("p (g j c) -> p g j c", g=4, j=NCH)
        wp_v = wp_sb.rearrange("p (j c) -> p j c", j=NCH)

        def vec8(name, d):
            t = const.tile([128, 8], F32, tag=name)
            nc.sync.dma_start(out=t, in_=d.ap().rearrange("(j p) -> p j", p=128))
            return t

        bq8 = vec8("bq8", bq_d)
        bk8 = vec8("bk8", bk_d)
        bp8 = vec8("bp8", bp_d)
        qg8 = vec8("qg8", qg_d)
        qb8 = vec8("qb8", qb_d)
        kg8 = vec8("kg8", kg_d)
        kb8 = vec8("kb8", kb_d)

        # ---- constants ----
        ones_blk = const.tile([128, 128], F32, tag="onesblk")
        nc.vector.memset(ones_blk, 1.0)
        ones1b = const.tile([128, 1], BF16, tag="ones1b")
        nc.vector.tensor_copy(out=ones1b, in_=ones_blk[:, 0:1])
        ones_row = const.tile([1, 128], BF16, tag="onesrow")
        nc.vector.tensor_copy(out=ones_row, in_=ones_blk[0:1, :])
        eps1 = const.tile([1, 1], F32)
        nc.vector.memset(eps1, EPS)
        scr1 = const.tile([1, 1], F32, tag="scr1")
        # dummy Ln at t=0 pre-loads the ln ACT table set off the critical path
        nc.scalar.activation(out=scr1, in_=eps1, func=AF.Ln, bias=eps1, scale=1.0)

        # persistent activations
        q_sb = qsbp.tile([128, NCH, TQ], BF16)      # q^T, later q-hat
        k_sb = ksbp.tile([128, NCH, T], BF16)       # k^T, later k-hat
        v_sb = vsbp.tile([128, NCH, H * 65], BF16)  # v head-interleaved + ones col
        outT_sb = osbp.tile([128, NCH, TQ], BF16)

        tmp_ctx = ExitStack()
        rows = tmp_ctx.enter_context(pool(name="rows", bufs=2))
        packp = tmp_ctx.enter_context(pool(name="pack", bufs=1))
        mrp = tmp_ctx.enter_context(pool(name="mr", bufs=1))
        sqp = tmp_ctx.enter_context(pool(name="sq", bufs=2))

        def rowchain_half(sum_ap, sq_ap, mu_out, d_out):
            """mu_out = sum/C (bf16); d_out = sumsq - sum*mu (f32), both [1,512]."""
            mu32 = rows.tile([1, 512], F32, tag="rmu")
            nc.vector.tensor_scalar(out=mu32, in0=sum_ap, scalar1=1.0 / C,
                                    scalar2=None, op0=OP.mult)
            nc.vector.tensor_copy(out=mu_out, in_=mu32)
            t2 = rows.tile([1, 512], F32, tag="rt2")
            nc.vector.tensor_tensor(out=t2, in0=sum_ap, in1=mu32, op=OP.mult)
            nc.vector.tensor_tensor(out=d_out, in0=sq_ap, in1=t2, op=OP.subtract)

        def ln_exp(d_row, rs_out):
            """rs_out = exp(-0.5 * ln(d/C + eps))."""
            nc.scalar.activation(out=d_row, in_=d_row, func=AF.Ln,
                                 bias=eps1, scale=1.0 / C)
            nc.scalar.activation(out=rs_out, in_=d_row, func=AF.Exp, scale=-0.5)

        def bc_half(mu_ap, rs_ap, bc_ps):
            """bc_ps[:, 0:512] = mu broadcast, [:, 512:1024] = rs broadcast."""
            nc.tensor.matmul(bc_ps[:, 0:512], ones_row, mu_ap, start=True, stop=True)
            nc.tensor.matmul(bc_ps[:, 512:1024], ones_row, rs_ap, start=True, stop=True)

        # ================= phase A: x stats, normalize =================
        qmm_ctx = ExitStack()
        qmmp = qmm_ctx.enter_context(pool(name="qmm", bufs=1, space="PSUM"))
        psA_ctx = ExitStack()
        psA = psA_ctx.enter_context(pool(name="psA", bufs=1, space="PSUM"))
        xstat_ps = psA.tile([97, 512], F32)
        for j in range(NCH):
            sqt = sqp.tile([128, T], BF16, tag="sqb")
            nc.vector.tensor_tensor(out=sqt, in0=xts[j], in1=xts[j], op=OP.mult)
            st, sp = j == 0, j == NCH - 1
            nc.tensor.matmul(xstat_ps[0:1, :], ones1b, xts[j][:, 0:512],
                             start=st, stop=sp, tile_position=(0, 0))
            nc.tensor.matmul(xstat_ps[32:33, :], ones1b, xts[j][:, 512:1024],
                             start=st, stop=sp, tile_position=(0, 32))
            nc.tensor.matmul(xstat_ps[64:65, :], ones1b, sqt[:, 0:512],
                             start=st, stop=sp, tile_position=(0, 64))
            nc.tensor.matmul(xstat_ps[96:97, :], ones1b, sqt[:, 512:1024],
                             start=st, stop=sp, tile_position=(0, 96))

        mux = packp.tile([1, 1024], BF16, tag="mux")   # mu0 | mu1
        rsx = packp.tile([1, 1024], BF16, tag="rsx")   # rs0 | rs1
        dx = rows.tile([1, 1024], F32, tag="rdx")
        rowchain_half(xstat_ps[0:1, :], xstat_ps[64:65, :], mux[:, 0:512], dx[:, 0:512])
        rowchain_half(xstat_ps[32:33, :], xstat_ps[96:97, :], mux[:, 512:1024], dx[:, 512:1024])
        ln_exp(dx, rsx)

        warm_ctx = ExitStack()
        warmp = warm_ctx.enter_context(pool(name="warm", bufs=1, space="PSUM"))
        warm_ps = warmp.tile([128, 512], F32)
        # warm matmul keyed on mux (written ~1us into the chain): keeps the
        # PE HAM activity window hot through the rowchain stall
        nc.tensor.matmul(warm_ps, ones_row, mux[:, 0:512], start=True, stop=True)

        bcx_ctx = ExitStack()
        bcxp = bcx_ctx.enter_context(pool(name="bcx", bufs=1, space="PSUM"))
        mrx = mrp.tile([128, 2048], BF16)  # mu0|rs0|mu1|rs1 bf16 broadcast
        for h in range(2):
            bch = bcxp.tile([128, 1024], F32, tag="bch")
            bc_half(mux[:, h * 512:(h + 1) * 512], rsx[:, h * 512:(h + 1) * 512], bch)
            nc.vector.tensor_copy(out=mrx[:, h * 1024:(h + 1) * 1024], in_=bch)

        # normalize in place, half 0 (query cols) first; q pass-1 matmuls are
        # interleaved j-outer so the PE consumes chunks as they land
        q_ps1 = qmmp.tile([128, 2048], F32, tag="mm")
        for j in range(NCH):
            tz = sqp.tile([128, T], BF16, tag="sqb")
            nc.vector.tensor_tensor(out=tz[:, 0:512], in0=xts[j][:, 0:512],
                                    in1=mrx[:, 0:512], op=OP.subtract)
            nc.vector.tensor_tensor(out=xts[j][:, 0:512], in0=tz[:, 0:512],
                                    in1=mrx[:, 512:1024], op=OP.mult)
            for m in range(4):
                nc.tensor.matmul(q_ps1[:, m * 512:(m + 1) * 512], wq_v[:, m, j, :],
                                 xts[j][:, 0:TQ], start=(j == 0), stop=(j == NCH - 1))
        for j in range(NCH):
            tz = sqp.tile([128, T], BF16, tag="sqb")
            nc.vector.tensor_tensor(out=tz[:, 0:512], in0=xts[j][:, 512:1024],
                                    in1=mrx[:, 1024:1536], op=OP.subtract)
            nc.vector.tensor_tensor(out=xts[j][:, 512:1024], in0=tz[:, 0:512],
                                    in1=mrx[:, 1536:2048], op=OP.mult)

        # ones columns of v (DVE, early; cols disjoint from v drains)
        v_ones_view = v_sb.rearrange("p i (h x) -> p i h x", x=65)[:, :, :, 64:65]
        nc.vector.tensor_copy(out=v_ones_view,
                              in_=ones_blk.rearrange("p (i h x) -> p i h x", i=NCH, h=H))

        bcx_ctx.close()
        warm_ctx.close()
        psA_ctx.close()

        # ================= phase B: q pass 2, q-LN, k, k-LN =================
        for m in range(4):
            nc.scalar.activation(out=q_sb[:, m, :], in_=q_ps1[:, m * 512:(m + 1) * 512],
                                 func=AF.Identity, bias=bq8[:, m:m + 1], scale=1.0)
        q_ps2 = qmmp.tile([128, 2048], F32, tag="mm")
        for j in range(NCH):
            for m in range(4):
                nc.tensor.matmul(q_ps2[:, m * 512:(m + 1) * 512], wq_v[:, 4 + m, j, :],
                                 xts[j][:, 0:TQ], start=(j == 0), stop=(j == NCH - 1))
        for m in range(4):
            nc.scalar.activation(out=q_sb[:, 4 + m, :], in_=q_ps2[:, m * 512:(m + 1) * 512],
                                 func=AF.Identity, bias=bq8[:, 4 + m:4 + m + 1], scale=1.0)
        qmm_ctx.close()

        # q stats (2-way col-tiled) + rowchain
        qs_ctx = ExitStack()
        qstatp = qs_ctx.enter_context(pool(name="qstat", bufs=1, space="PSUM"))
        qstat_ps = qstatp.tile([33, TQ], F32)
        for m in range(NCH):
            sqt = sqp.tile([128, T], BF16, tag="sqb")
            nc.vector.tensor_tensor(out=sqt[:, 0:TQ], in0=q_sb[:, m, :], in1=q_sb[:, m, :],
                                    op=OP.mult)
            st, sp = m == 0, m == NCH - 1
            nc.tensor.matmul(qstat_ps[0:1, :], ones1b, q_sb[:, m, :],
                             start=st, stop=sp, tile_position=(0, 0))
            nc.tensor.matmul(qstat_ps[32:33, :], ones1b, sqt[:, 0:TQ],
                             start=st, stop=sp, tile_position=(0, 32))

        kmm_ctx = ExitStack()
        kmmp = kmm_ctx.enter_context(pool(name="kmm", bufs=2, space="PSUM"))

        muq = packp.tile([1, 512], BF16, tag="muq")
        rsq = packp.tile([1, 512], BF16, tag="rsq")
        dq = rows.tile([1, 512], F32, tag="rdq")
        rowchain_half(qstat_ps[0:1, :], qstat_ps[32:33, :], muq, dq)
        ln_exp(dq, rsq)

        bcq_ctx = ExitStack()
        bcqp = bcq_ctx.enter_context(pool(name="bcq", bufs=1, space="PSUM"))
        mrq = mrp.tile([128, 1024], BF16, tag="mrq")

        for m in range(NCH):
            k_ps = kmmp.tile([128, T], F32, tag="mm")
            for n in range(2):
                for j in range(NCH):
                    nc.tensor.matmul(k_ps[:, n * 512:(n + 1) * 512], wk_v[:, m, j, :],
                                     xts[j][:, n * 512:(n + 1) * 512],
                                     start=(j == 0), stop=(j == NCH - 1))
            nc.scalar.activation(out=k_sb[:, m, :], in_=k_ps, func=AF.Identity,
                                 bias=bk8[:, m:m + 1], scale=1.0)
            if m == 0:
                # q broadcast right after k[0]'s matmuls; q-hat applies overlap
                # the remaining k matmuls
                bcq0 = bcqp.tile([128, 1024], F32, tag="bq")
                bc_half(muq, rsq, bcq0)
                nc.vector.tensor_copy(out=mrq, in_=bcq0)
                for mm_ in range(NCH):
                    t1 = sqp.tile([128, T], BF16, tag="sqb")
                    nc.vector.tensor_tensor(out=t1[:, 0:TQ], in0=q_sb[:, mm_, :],
                                            in1=mrq[:, 0:512], op=OP.subtract)
                    t2 = sqp.tile([128, T], BF16, tag="sqb")
                    nc.vector.tensor_tensor(out=t2[:, 0:TQ], in0=t1[:, 0:TQ],
                                            in1=mrq[:, 512:1024], op=OP.mult)
                    nc.scalar.activation(out=q_sb[:, mm_, :], in_=t2[:, 0:TQ],
                                         func=AF.Identity, bias=qb8[:, mm_:mm_ + 1],
                                         scale=qg8[:, mm_:mm_ + 1])

        bcq_ctx.close()
        kmm_ctx.close()
        qs_ctx.close()

        # k stats (4-way col-tiled)
        ks_ctx = ExitStack()
        kstatp = ks_ctx.enter_context(pool(name="kstat", bufs=1, space="PSUM"))
        kstat_ps = kstatp.tile([97, 512], F32)
        for m in range(NCH):
            sqt = sqp.tile([128, T], BF16, tag="sqb")
            nc.vector.tensor_tensor(out=sqt, in0=k_sb[:, m, :], in1=k_sb[:, m, :], op=OP.mult)
            st, sp = m == 0, m == NCH - 1
            nc.tensor.matmul(kstat_ps[0:1, :], ones1b, k_sb[:, m, 0:512],
                             start=st, stop=sp, tile_position=(0, 0))
            nc.tensor.matmul(kstat_ps[32:33, :], ones1b, k_sb[:, m, 512:1024],
                             start=st, stop=sp, tile_position=(0, 32))
            nc.tensor.matmul(kstat_ps[64:65, :], ones1b, sqt[:, 0:512],
                             start=st, stop=sp, tile_position=(0, 64))
            nc.tensor.matmul(kstat_ps[96:97, :], ones1b, sqt[:, 512:1024],
                             start=st, stop=sp, tile_position=(0, 96))

        # ================= phase C: v matmuls, then attention =================
        vps_ctx = ExitStack()
        vpsp = vps_ctx.enter_context(pool(name="vps", bufs=2, space="PSUM"))

        muk = packp.tile([1, 1024], BF16, tag="muk")
        rsk = packp.tile([1, 1024], BF16, tag="rsk")
        dk = rows.tile([1, 1024], F32, tag="rdk")
        rowchain_half(kstat_ps[0:1, :], kstat_ps[64:65, :], muk[:, 0:512], dk[:, 0:512])
        rowchain_half(kstat_ps[32:33, :], kstat_ps[96:97, :], muk[:, 512:1024], dk[:, 512:1024])
        ln_exp(dk, rsk)

        bck_ctx = ExitStack()
        bckp = bck_ctx.enter_context(pool(name="bck", bufs=1, space="PSUM"))
        mrk = mrp.tile([128, 2048], BF16, tag="mrk")

        def vgroup(g):
            wvsl = wv_v[:, g]
            for i in range(NCH):
                v_ps = vpsp.tile([128, 256], F32, tag="vps")
                for j in range(NCH):
                    nc.tensor.matmul(v_ps, xts[j][:, i * 128:(i + 1) * 128],
                                     wvsl[:, j, :], start=(j == 0), stop=(j == NCH - 1))
                vout = v_sb.rearrange("p i (h x) -> p i h x", x=65)[:, i, g * 4:(g + 1) * 4, 0:64]
                nc.vector.tensor_copy(out=vout, in_=v_ps.rearrange("p (h x) -> p h x", x=64))

        vgroup(0)
        # k broadcast + k-hat apply, hidden under the v matmuls
        for h in range(2):
            bckh = bckp.tile([128, 1024], F32, tag="bk")
            bc_half(muk[:, h * 512:(h + 1) * 512], rsk[:, h * 512:(h + 1) * 512], bckh)
            nc.vector.tensor_copy(out=mrk[:, h * 1024:(h + 1) * 1024], in_=bckh)
        vgroup(1)

        for m in range(NCH):
            t1 = sqp.tile([128, T], BF16, tag="sqb")
            for h in range(2):
                nc.vector.tensor_tensor(out=t1[:, h * 512:(h + 1) * 512],
                                        in0=k_sb[:, m, h * 512:(h + 1) * 512],
                                        in1=mrk[:, h * 1024:h * 1024 + 512], op=OP.subtract)
            t2 = sqp.tile([128, T], BF16, tag="sqb")
            for h in range(2):
                nc.vector.tensor_tensor(out=t2[:, h * 512:(h + 1) * 512],
                                        in0=t1[:, h * 512:(h + 1) * 512],
                                        in1=mrk[:, h * 1024 + 512:h * 1024 + 1024], op=OP.mult)
            nc.scalar.activation(out=k_sb[:, m, :], in_=t2,
                                 func=AF.Identity, bias=kb8[:, m:m + 1],
                                 scale=kg8[:, m:m + 1])

        vgroup(2)
        vgroup(3)

        bck_ctx.close()
        vps_ctx.close()
        ks_ctx.close()
        tmp_ctx.close()
        xz_ctx.close()

        # ---- attention: software-pipelined pairs ----
        att_ctx = ExitStack()
        pexpp = att_ctx.enter_context(pool(name="pexp", bufs=8))
        denp = att_ctx.enter_context(pool(name="den", bufs=1))
        rcbp = att_ctx.enter_context(pool(name="rcb", bufs=2))
        scp = att_ctx.enter_context(pool(name="sc", bufs=1, space="PSUM"))
        avp = att_ctx.enter_context(pool(name="av", bufs=3, space="PSUM"))

        p_tiles = {}
        av_tiles = {}

        def sc_group(m, g2, av_mm=None):
            """scores+exp for pair m, chunks (2*g2, 2*g2+1); av_mm emits the
            interleaved av matmuls for the previous pair."""
            sc_ps = scp.tile([128, 2048], F32, tag="sc")
            for u in range(2):
                i = 2 * g2 + u
                nc.tensor.matmul(sc_ps[:, u * 1024:u * 1024 + 512],
                                 k_sb[0:64, m, i * 128:(i + 1) * 128],
                                 q_sb[0:64, m, :], start=True, stop=True)
                nc.tensor.matmul(sc_ps[:, u * 1024 + 512:u * 1024 + 1024],
                                 k_sb[64:128, m, i * 128:(i + 1) * 128],
                                 q_sb[64:128, m, :], start=True, stop=True)
            p_sb = pexpp.tile([128, 2048], BF16, tag="p")
            nc.scalar.activation(out=p_sb, in_=sc_ps, func=AF.Exp, scale=0.125)
            p_tiles[m].append(p_sb)
            if av_mm is not None:
                av_mm(g2)

        def av_group_fn(mprev):
            p_list = p_tiles[mprev]
            h0, h1 = 2 * mprev, 2 * mprev + 1
            av0 = avp.tile([65, TQ], F32, tag="av")
            av1 = avp.tile([65, TQ], F32, tag="av")
            av_tiles[mprev] = (av0, av1)

            def av_mm(g2):
                for u in range(2):
                    i = 2 * g2 + u
                    st, sp = i == 0, i == NCH - 1
                    nc.tensor.matmul(av0, v_sb[:, i, h0 * 65:h0 * 65 + 65],
                                     p_list[g2][:, u * 1024:u * 1024 + 512],
                                     start=st, stop=sp)
                    nc.tensor.matmul(av1, v_sb[:, i, h1 * 65:h1 * 65 + 65],
                                     p_list[g2][:, u * 1024 + 512:u * 1024 + 1024],
                                     start=st, stop=sp)
            return av_mm

        def av_drain(mprev):
            av0, av1 = av_tiles.pop(mprev)
            p_tiles.pop(mprev)
            dd = denp.tile([1, 2 * TQ], F32, tag="den")
            nc.vector.tensor_copy(out=dd[:, 0:TQ], in_=av0[64:65, :])
            nc.vector.tensor_copy(out=dd[:, TQ:2 * TQ], in_=av1[64:65, :])
            rt = denp.tile([1, 2 * TQ], F32, tag="rect")
            nc.vector.reciprocal_approx_fast(out=rt, in_=dd)
            rbb = rcbp.tile([64, 2 * TQ], F32, tag="rbb")
            nc.gpsimd.partition_broadcast(rbb, rt)
            nc.vector.tensor_tensor(out=outT_sb[0:64, mprev, :],
                                    in0=av0[0:64, :], in1=rbb[:, 0:TQ], op=OP.mult)
            nc.vector.tensor_tensor(out=outT_sb[64:128, mprev, :],
                                    in0=av1[0:64, :], in1=rbb[:, TQ:2 * TQ], op=OP.mult)

        p_tiles[0] = []
        for g2 in range(4):
            sc_group(0, g2)
        for m in range(1, NCH):
            p_tiles[m] = []
            av_mm = av_group_fn(m - 1)
            for g2 in range(4):
                sc_group(m, g2, av_mm)
            av_drain(m - 1)
        av_mm = av_group_fn(NCH - 1)
        for g2 in range(4):
            av_mm(g2)
        av_drain(NCH - 1)

        att_ctx.close()

        # ================= phase D: proj =================
        youtp = ctx.enter_context(pool(name="yout", bufs=2))
        pjp = ctx.enter_context(pool(name="pj", bufs=2, space="PSUM"))
        for m in range(NCH):
            y_ps = pjp.tile([128, TQ], F32, tag="pj")
            for j in range(NCH):
                nc.tensor.matmul(y_ps, wp_v[:, j, m * 128:(m + 1) * 128], outT_sb[:, j, :],
                                 start=(j == 0), stop=(j == NCH - 1))
            y_sb = youtp.tile([128, TQ], F32, tag="y")
            nc.scalar.activation(out=y_sb, in_=y_ps, func=AF.Identity,
                                 bias=bp8[:, m:m + 1], scale=1.0)
            nc.sync.dma_start(out=yT_d[m * 128:(m + 1) * 128, :], in_=y_sb)

    nc.finalize()
    return nc


def _get_nc():
    if "nc" not in _CACHE:
        _CACHE["nc"] = _build()
    return _CACHE["nc"]


def _lay_w(w, gcols):
    """[C, C] -> [128, 8192] slab-contiguous: A[p, m, j, c'] = w[j*128+p, m*gcols+c']."""
    A = w.reshape(NCH, 128, C // gcols, gcols).transpose(1, 2, 0, 3)
    return np.ascontiguousarray(A.reshape(128, NCH * C))


def _prep_inputs(x, norm_g, norm_b, qkv_w, qkv_b, qln_g, qln_b, kln_g, kln_b, proj_w, proj_b):
    x = np.asarray(x, dtype=np.float32)
    norm_g = np.asarray(norm_g, dtype=np.float32)
    norm_b = np.asarray(norm_b, dtype=np.float32)
    qkv_w = np.asarray(qkv_w, dtype=np.float32)
    qkv_b = np.asarray(qkv_b, dtype=np.float32)
    proj_w = np.asarray(proj_w, dtype=np.float32)
    proj_b = np.asarray(proj_b, dtype=np.float32)

    wfold = norm_g[:, None] * qkv_w                    # [C, 3C]
    bfold = qkv_b + norm_b @ qkv_w                     # [3C]
    wq = np.ascontiguousarray(wfold[:, 0:C])
    wk = np.ascontiguousarray(wfold[:, C:2 * C])
    wv = np.ascontiguousarray(wfold[:, 2 * C:3 * C])
    bq, bk, bv = bfold[0:C].copy(), bfold[C:2 * C].copy(), bfold[2 * C:3 * C].copy()
    # v bias folds through attention (softmax rows sum to 1) into proj bias
    bp = proj_b + bv @ proj_w

    bf16 = ml_dtypes.bfloat16
    wp_lay = np.ascontiguousarray(
        proj_w.reshape(NCH, 128, C).transpose(1, 0, 2).reshape(128, NCH * C))
    common = dict(
        wq=_lay_w(wq, 128).astype(bf16), wk=_lay_w(wk, 128).astype(bf16),
        wv=_lay_w(wv, 256).astype(bf16), wp=wp_lay.astype(bf16),
        bq=bq, bk=bk, bp=bp,
        qg=np.asarray(qln_g, dtype=np.float32).copy(),
        qb=np.asarray(qln_b, dtype=np.float32).copy(),
        kg=np.asarray(kln_g, dtype=np.float32).copy(),
        kb=np.asarray(kln_b, dtype=np.float32).copy(),
    )
    in_maps = []
    for core in range(8):
        b, half = core // 2, core % 2
        xp = np.concatenate([x[b, TQ * half:], x[b, :TQ * half]], axis=0) if half else x[b]
        xT = np.ascontiguousarray(xp.T).astype(bf16)
        in_maps.append(dict(common, xT=xT))
    return in_maps


def kernel(**inputs) -> np.ndarray:
    in_maps = _prep_inputs(**inputs)
    nc = _get_nc()
    res = run_bass_kernel_spmd(nc, in_maps, core_ids=list(range(8)))
    out = np.empty((B, T, C), dtype=np.float32)
    for core in range(8):
        b, half = core // 2, core % 2
        out[b, TQ * half:TQ * half + TQ, :] = res.results[core]["yT"].T
    return out


# revision 12
# speedup vs baseline: 1.1737x; 1.1479x over previous
"""Trainium2 Bass kernel for the pre-LN multi-head attention block.

Sharding: 8 cores = 4 batches x 2 query-row halves, collective-free. Each core
computes all 16 heads for its 512 query rows, with full-T k/v for its batch
(k/v compute duplicated across the 2 cores of a batch).

Per-core scheme (C=1024 channels, T=1024 rows, TQ=512 query rows):
  - everything is bf16 into the PE; PSUM accumulates fp32. Host pre-casts x^T
    and all weights to bf16 and lays the weights out slab-contiguous so each
    weight matrix is ONE [128, 8192] DMA (16KB contiguous per partition).
  - LN stats via bf16 ones-matmuls, column-tiled per 512-col half; the
    mean/rstd rowchain runs on 512-wide rows, with ONE batched Ln and ONE Exp
    per LN phase (minimizes ACT table-set switches); rows are broadcast
    across partitions with K=1 ones-matmuls on the PE then one DVE copy to
    bf16 SBUF; normalize = 2 bf16 DVE ops per chunk-half, half 0 first.
  - q matmuls run j-outer in two 4-output-chunk passes (4 PSUM banks each)
    so they overlap the tail of the x normalize.
  - v bias is folded into the proj bias on the host (bp' = bp + bv @ Wp), so
    v PSUM drains are plain copies; v psum is double-buffered. All v matmuls
    run before attention so attention is exp/ACT-bound.
  - scores^T per head pair = 2 matmuls (K=64 halves) which the PE runs
    concurrently via row-group tiling; exp on ACT over 2-chunk [128, 2048]
    groups (scale=0.125 folded in); p stored bf16.
  - attention is software-pipelined: score/exp groups of pair m interleave
    with the attn@v matmuls of pair m-1, so the PE has av work while exps
    pace the pipeline.
  - attn@v: both heads via 65-col augmented v (ones col -> denominator row);
    denominators: psum row 64 -> SBUF, reciprocal_approx_fast, GpSimd
    partition_broadcast, then the av PSUM drain fuses the 1/den scaling.
  - proj: y^T = Wp^T out^T + bias'; double-buffered psum; host transposes.
"""

from contextlib import ExitStack

import ml_dtypes
import numpy as np

import concourse.bacc as bacc
import concourse.mybir as mybir
import concourse.tile as tile
from concourse.bass_utils import run_bass_kernel_spmd

F32 = mybir.dt.float32
BF16 = mybir.dt.bfloat16
AF = mybir.ActivationFunctionType
OP = mybir.AluOpType

B, T, C = 4, 1024, 1024
H, D = 16, 64
TQ = 512           # query rows per core
NCH = 8            # 128-row chunks of C (or T)
EPS = 1e-5

_CACHE = {}


def _build():
    nc = bacc.Bacc(None, target_bir_lowering=False, debug=False)

    xT_d = nc.declare_dram_parameter("xT", [C, T], BF16, isOutput=False)
    wq_d = nc.declare_dram_parameter("wq", [128, NCH * C], BF16, isOutput=False)
    wk_d = nc.declare_dram_parameter("wk", [128, NCH * C], BF16, isOutput=False)
    wv_d = nc.declare_dram_parameter("wv", [128, NCH * C], BF16, isOutput=False)
    wp_d = nc.declare_dram_parameter("wp", [128, NCH * C], BF16, isOutput=False)
    bq_d = nc.declare_dram_parameter("bq", [C], F32, isOutput=False)
    bk_d = nc.declare_dram_parameter("bk", [C], F32, isOutput=False)
    bp_d = nc.declare_dram_parameter("bp", [C], F32, isOutput=False)
    qg_d = nc.declare_dram_parameter("qg", [C], F32, isOutput=False)
    qb_d = nc.declare_dram_parameter("qb", [C], F32, isOutput=False)
    kg_d = nc.declare_dram_parameter("kg", [C], F32, isOutput=False)
    kb_d = nc.declare_dram_parameter("kb", [C], F32, isOutput=False)
    yT_d = nc.declare_dram_parameter("yT", [C, TQ], F32, isOutput=True)

    with tile.TileContext(nc) as tc, ExitStack() as ctx:
        pool = tc.tile_pool

        const = ctx.enter_context(pool(name="const", bufs=1))
        wqp = ctx.enter_context(pool(name="wqp", bufs=1))
        wkp = ctx.enter_context(pool(name="wkp", bufs=1))
        wvp = ctx.enter_context(pool(name="wvp", bufs=1))
        wpp = ctx.enter_context(pool(name="wpp", bufs=1))
        qsbp = ctx.enter_context(pool(name="qsb", bufs=1))
        ksbp = ctx.enter_context(pool(name="ksb", bufs=1))
        vsbp = ctx.enter_context(pool(name="vsb", bufs=1))
        osbp = ctx.enter_context(pool(name="osb", bufs=1))

        # ============ big-load FIFO: x chunks, then all weights ============
        xz_ctx = ExitStack()
        xzp = xz_ctx.enter_context(pool(name="xz", bufs=1))
        xts = []
        for j in range(NCH):
            t = xzp.tile([128, T], BF16, tag=f"x{j}")
            nc.sync.dma_start(out=t, in_=xT_d[j * 128:(j + 1) * 128, :])
            xts.append(t)

        wq_sb = wqp.tile([128, NCH * C], BF16)
        nc.sync.dma_start(out=wq_sb, in_=wq_d.ap())
        wk_sb = wkp.tile([128, NCH * C], BF16)
        nc.sync.dma_start(out=wk_sb, in_=wk_d.ap())
        wv_sb = wvp.tile([128, NCH * C], BF16)
        nc.sync.dma_start(out=wv_sb, in_=wv_d.ap())
        wp_sb = wpp.tile([128, NCH * C], BF16)
        nc.sync.dma_start(out=wp_sb, in_=wp_d.ap())

        wq_v = wq_sb.rearrange("p (m j c) -> p m j c", m=NCH, j=NCH)
        wk_v = wk_sb.rearrange("p (m j c) -> p m j c", m=NCH, j=NCH)
        wv_v = wv_sb.rearrange("p (g j c) -> p g j c", g=4, j=NCH)
        wp_v = wp_sb.rearrange("p (j c) -> p j c", j=NCH)

        def vec8(name, d):
            t = const.tile([128, 8], F32, tag=name)
            nc.sync.dma_start(out=t, in_=d.ap().rearrange("(j p) -> p j", p=128))
            return t

        bq8 = vec8("bq8", bq_d)
        bk8 = vec8("bk8", bk_d)
        bp8 = vec8("bp8", bp_d)
        qg8 = vec8("qg8", qg_d)
        qb8 = vec8("qb8", qb_d)
        kg8 = vec8("kg8", kg_d)
        kb8 = vec8("kb8", kb_d)

        # ---- constants ----
        ones_blk = const.tile([128, 128], F32, tag="onesblk")
        nc.vector.memset(ones_blk, 1.0)
        ones1b = const.tile([128, 1], BF16, tag="ones1b")
        nc.vector.tensor_copy(out=ones1b, in_=ones_blk[:, 0:1])
        ones_row = const.tile([1, 128], BF16, tag="onesrow")
        nc.vector.tensor_copy(out=ones_row, in_=ones_blk[0:1, :])
        eps1 = const.tile([1, 1], F32)
        nc.vector.memset(eps1, EPS)
        scr1 = const.tile([1, 1], F32, tag="scr1")
        # dummy Sqrt at t=0 pre-loads the sqrt ACT table set off the critical path
        nc.scalar.activation(out=scr1, in_=eps1, func=AF.Sqrt, bias=eps1, scale=1.0)

        # persistent activations
        q_sb = qsbp.tile([128, NCH, TQ], BF16)      # q^T, later q-hat
        k_sb = ksbp.tile([128, NCH, T], BF16)       # k^T, later k-hat
        v_sb = vsbp.tile([128, NCH, H * 65], BF16)  # v head-interleaved + ones col
        outT_sb = osbp.tile([128, NCH, TQ], BF16)

        tmp_ctx = ExitStack()
        rows = tmp_ctx.enter_context(pool(name="rows", bufs=2))
        packp = tmp_ctx.enter_context(pool(name="pack", bufs=1))
        mrp = tmp_ctx.enter_context(pool(name="mr", bufs=1))
        sqp = tmp_ctx.enter_context(pool(name="sq", bufs=2))
        qsqp = tmp_ctx.enter_context(pool(name="qsq", bufs=1))

        def rowchain_half(sum_ap, sq_ap, mu_out, d_out):
            """mu_out = sum/C (bf16); d_out = sumsq - sum*mu (f32), both [1,512]."""
            mu32 = rows.tile([1, 512], F32, tag="rmu")
            nc.vector.tensor_scalar(out=mu32, in0=sum_ap, scalar1=1.0 / C,
                                    scalar2=None, op0=OP.mult)
            nc.vector.tensor_copy(out=mu_out, in_=mu32)
            t2 = rows.tile([1, 512], F32, tag="rt2")
            nc.vector.tensor_tensor(out=t2, in0=sum_ap, in1=mu32, op=OP.mult)
            nc.vector.tensor_tensor(out=d_out, in0=sq_ap, in1=t2, op=OP.subtract)

        def ln_exp(d_row, rs_out):
            """rs_out = 1/sqrt(d/C + eps): ACT Sqrt (sqrt stays the resident
            table set through all three LN phases) + DVE fast reciprocal."""
            nc.scalar.activation(out=d_row, in_=d_row, func=AF.Sqrt,
                                 bias=eps1, scale=1.0 / C)
            rcp = rows.tile([1, 1024], F32, tag="rrcp")
            n = d_row.shape[-1]
            nc.vector.reciprocal_approx_fast(out=rcp[:, 0:n], in_=d_row)
            nc.vector.tensor_copy(out=rs_out, in_=rcp[:, 0:n])

        def bc_half(mu_ap, rs_ap, bc_ps):
            """bc_ps[:, 0:512] = mu broadcast, [:, 512:1024] = rs broadcast."""
            nc.tensor.matmul(bc_ps[:, 0:512], ones_row, mu_ap, start=True, stop=True)
            nc.tensor.matmul(bc_ps[:, 512:1024], ones_row, rs_ap, start=True, stop=True)

        # ================= phase A: x stats, normalize =================
        qmm_ctx = ExitStack()
        qmmp = qmm_ctx.enter_context(pool(name="qmm", bufs=1, space="PSUM"))
        psA_ctx = ExitStack()
        psA = psA_ctx.enter_context(pool(name="psA", bufs=1, space="PSUM"))
        xstat_ps = psA.tile([97, 512], F32)
        for j in range(NCH):
            sqt = sqp.tile([128, T], BF16, tag="sqb")
            nc.vector.tensor_tensor(out=sqt, in0=xts[j], in1=xts[j], op=OP.mult)
            st, sp = j == 0, j == NCH - 1
            nc.tensor.matmul(xstat_ps[0:1, :], ones1b, xts[j][:, 0:512],
                             start=st, stop=sp, tile_position=(0, 0))
            nc.tensor.matmul(xstat_ps[32:33, :], ones1b, xts[j][:, 512:1024],
                             start=st, stop=sp, tile_position=(0, 32))
            nc.tensor.matmul(xstat_ps[64:65, :], ones1b, sqt[:, 0:512],
                             start=st, stop=sp, tile_position=(0, 64))
            nc.tensor.matmul(xstat_ps[96:97, :], ones1b, sqt[:, 512:1024],
                             start=st, stop=sp, tile_position=(0, 96))

        mux = packp.tile([1, 1024], BF16, tag="mux")   # mu0 | mu1
        rsx = packp.tile([1, 1024], BF16, tag="rsx")   # rs0 | rs1
        dx = rows.tile([1, 1024], F32, tag="rdx")
        rowchain_half(xstat_ps[0:1, :], xstat_ps[64:65, :], mux[:, 0:512], dx[:, 0:512])
        rowchain_half(xstat_ps[32:33, :], xstat_ps[96:97, :], mux[:, 512:1024], dx[:, 512:1024])
        ln_exp(dx, rsx)

        warm_ctx = ExitStack()
        warmp = warm_ctx.enter_context(pool(name="warm", bufs=1, space="PSUM"))
        warm_ps = warmp.tile([128, 512], F32)
        # warm matmul keyed on mux (written ~1us into the chain): keeps the
        # PE HAM activity window hot through the rowchain stall
        nc.tensor.matmul(warm_ps, ones_row, mux[:, 0:512], start=True, stop=True)

        bcx_ctx = ExitStack()
        bcxp = bcx_ctx.enter_context(pool(name="bcx", bufs=1, space="PSUM"))
        mrx = mrp.tile([128, 2048], BF16)  # mu0|rs0|mu1|rs1 bf16 broadcast
        for h in range(2):
            bch = bcxp.tile([128, 1024], F32, tag="bch")
            bc_half(mux[:, h * 512:(h + 1) * 512], rsx[:, h * 512:(h + 1) * 512], bch)
            nc.vector.tensor_copy(out=mrx[:, h * 1024:(h + 1) * 1024], in_=bch)

        # normalize in place, half 0 (query cols) first; q pass-1 matmuls are
        # interleaved j-outer so the PE consumes chunks as they land
        q_ps1 = qmmp.tile([128, 2048], F32, tag="mm")
        for j in range(NCH):
            tz = sqp.tile([128, T], BF16, tag="sqb")
            nc.vector.tensor_tensor(out=tz[:, 0:512], in0=xts[j][:, 0:512],
                                    in1=mrx[:, 0:512], op=OP.subtract)
            nc.vector.tensor_tensor(out=xts[j][:, 0:512], in0=tz[:, 0:512],
                                    in1=mrx[:, 512:1024], op=OP.mult)
            for m in range(4):
                nc.tensor.matmul(q_ps1[:, m * 512:(m + 1) * 512], wq_v[:, m, j, :],
                                 xts[j][:, 0:TQ], start=(j == 0), stop=(j == NCH - 1))
        for j in range(NCH):
            tz = sqp.tile([128, T], BF16, tag="sqb")
            nc.vector.tensor_tensor(out=tz[:, 0:512], in0=xts[j][:, 512:1024],
                                    in1=mrx[:, 1024:1536], op=OP.subtract)
            nc.vector.tensor_tensor(out=xts[j][:, 512:1024], in0=tz[:, 0:512],
                                    in1=mrx[:, 1536:2048], op=OP.mult)

        # ones columns of v (DVE, early; cols disjoint from v drains)
        v_ones_view = v_sb.rearrange("p i (h x) -> p i h x", x=65)[:, :, :, 64:65]
        nc.vector.tensor_copy(out=v_ones_view,
                              in_=ones_blk.rearrange("p (i h x) -> p i h x", i=NCH, h=H))

        bcx_ctx.close()
        warm_ctx.close()
        psA_ctx.close()

        # ================= phase B: q pass 2, q-LN, k, k-LN =================
        for m in range(4):
            nc.scalar.activation(out=q_sb[:, m, :], in_=q_ps1[:, m * 512:(m + 1) * 512],
                                 func=AF.Identity, bias=bq8[:, m:m + 1], scale=1.0)
        q_ps2 = qmmp.tile([128, 2048], F32, tag="mm")
        for j in range(NCH):
            for m in range(4):
                nc.tensor.matmul(q_ps2[:, m * 512:(m + 1) * 512], wq_v[:, 4 + m, j, :],
                                 xts[j][:, 0:TQ], start=(j == 0), stop=(j == NCH - 1))
        for m in range(4):
            nc.scalar.activation(out=q_sb[:, 4 + m, :], in_=q_ps2[:, m * 512:(m + 1) * 512],
                                 func=AF.Identity, bias=bq8[:, 4 + m:4 + m + 1], scale=1.0)
        qmm_ctx.close()

        # q stats: squares on DVE now; the ones-matmuls are deferred into the
        # k loop so the PE is not stalled waiting on q drains
        qs_ctx = ExitStack()
        qstatp = qs_ctx.enter_context(pool(name="qstat", bufs=1, space="PSUM"))
        qstat_ps = qstatp.tile([33, TQ], F32)
        qsq = []
        for m in range(NCH):
            sqt = qsqp.tile([128, TQ], BF16, tag=f"qsq{m}")
            nc.vector.tensor_tensor(out=sqt, in0=q_sb[:, m, :], in1=q_sb[:, m, :],
                                    op=OP.mult)
            qsq.append(sqt)

        ks_ctx = ExitStack()
        kstatp = ks_ctx.enter_context(pool(name="kstat", bufs=1, space="PSUM"))
        kmm_ctx = ExitStack()
        kmmp = kmm_ctx.enter_context(pool(name="kmm", bufs=2, space="PSUM"))

        muq = packp.tile([1, 512], BF16, tag="muq")
        rsq = packp.tile([1, 512], BF16, tag="rsq")
        dq = rows.tile([1, 512], F32, tag="rdq")

        bcq_ctx = ExitStack()
        bcqp = bcq_ctx.enter_context(pool(name="bcq", bufs=1, space="PSUM"))
        mrq = mrp.tile([128, 1024], BF16, tag="mrq")

        kstat_ps = kstatp.tile([97, 512], F32)

        def kstat_mm(m):
            sqt = sqp.tile([128, T], BF16, tag="ksq")
            nc.vector.tensor_tensor(out=sqt, in0=k_sb[:, m, :], in1=k_sb[:, m, :], op=OP.mult)
            st, sp = m == 0, m == NCH - 1
            nc.tensor.matmul(kstat_ps[0:1, :], ones1b, k_sb[:, m, 0:512],
                             start=st, stop=sp, tile_position=(0, 0))
            nc.tensor.matmul(kstat_ps[32:33, :], ones1b, k_sb[:, m, 512:1024],
                             start=st, stop=sp, tile_position=(0, 32))
            nc.tensor.matmul(kstat_ps[64:65, :], ones1b, sqt[:, 0:512],
                             start=st, stop=sp, tile_position=(0, 64))
            nc.tensor.matmul(kstat_ps[96:97, :], ones1b, sqt[:, 512:1024],
                             start=st, stop=sp, tile_position=(0, 96))

        for m in range(NCH):
            k_ps = kmmp.tile([128, T], F32, tag="mm")
            for n in range(2):
                for j in range(NCH):
                    nc.tensor.matmul(k_ps[:, n * 512:(n + 1) * 512], wk_v[:, m, j, :],
                                     xts[j][:, n * 512:(n + 1) * 512],
                                     start=(j == 0), stop=(j == NCH - 1))
            nc.scalar.activation(out=k_sb[:, m, :], in_=k_ps, func=AF.Identity,
                                 bias=bk8[:, m:m + 1], scale=1.0)
            if m == 1:
                # q stat ones-matmuls: q drains/squares are done by now
                for mm_ in range(NCH):
                    st, sp = mm_ == 0, mm_ == NCH - 1
                    nc.tensor.matmul(qstat_ps[0:1, :], ones1b, q_sb[:, mm_, :],
                                     start=st, stop=sp, tile_position=(0, 0))
                    nc.tensor.matmul(qstat_ps[32:33, :], ones1b, qsq[mm_],
                                     start=st, stop=sp, tile_position=(0, 32))
                rowchain_half(qstat_ps[0:1, :], qstat_ps[32:33, :], muq, dq)
                ln_exp(dq, rsq)
            if m == 2:
                bcq0 = bcqp.tile([128, 1024], F32, tag="bq")
                bc_half(muq, rsq, bcq0)
                nc.vector.tensor_copy(out=mrq, in_=bcq0)
                for mm_ in range(NCH):
                    t1 = sqp.tile([128, T], BF16, tag="sqb")
                    nc.vector.tensor_tensor(out=t1[:, 0:TQ], in0=q_sb[:, mm_, :],
                                            in1=mrq[:, 0:512], op=OP.subtract)
                    t2 = sqp.tile([128, T], BF16, tag="sqb")
                    nc.vector.tensor_tensor(out=t2[:, 0:TQ], in0=t1[:, 0:TQ],
                                            in1=mrq[:, 512:1024], op=OP.mult)
                    nc.scalar.activation(out=q_sb[:, mm_, :], in_=t2[:, 0:TQ],
                                         func=AF.Identity, bias=qb8[:, mm_:mm_ + 1],
                                         scale=qg8[:, mm_:mm_ + 1])
            if m >= 3:
                # staggered k stats for chunk m-3 (its drain+square are done)
                kstat_mm(m - 3)
        for m in range(NCH - 3, NCH):
            kstat_mm(m)

        bcq_ctx.close()
        kmm_ctx.close()

        # ================= phase C: v matmuls, then attention =================
        vps_ctx = ExitStack()
        vpsp = vps_ctx.enter_context(pool(name="vps", bufs=2, space="PSUM"))

        muk = packp.tile([1, 1024], BF16, tag="muk")
        rsk = packp.tile([1, 1024], BF16, tag="rsk")
        dk = rows.tile([1, 1024], F32, tag="rdk")
        rowchain_half(kstat_ps[0:1, :], kstat_ps[64:65, :], muk[:, 0:512], dk[:, 0:512])
        rowchain_half(kstat_ps[32:33, :], kstat_ps[96:97, :], muk[:, 512:1024], dk[:, 512:1024])
        ln_exp(dk, rsk)

        bck_ctx = ExitStack()
        bckp = bck_ctx.enter_context(pool(name="bck", bufs=1, space="PSUM"))
        mrk = mrp.tile([128, 2048], BF16, tag="mrk")

        def vgroup(g):
            wvsl = wv_v[:, g]
            for i in range(NCH):
                v_ps = vpsp.tile([128, 256], F32, tag="vps")
                for j in range(NCH):
                    nc.tensor.matmul(v_ps, xts[j][:, i * 128:(i + 1) * 128],
                                     wvsl[:, j, :], start=(j == 0), stop=(j == NCH - 1))
                vout = v_sb.rearrange("p i (h x) -> p i h x", x=65)[:, i, g * 4:(g + 1) * 4, 0:64]
                nc.vector.tensor_copy(out=vout, in_=v_ps.rearrange("p (h x) -> p h x", x=64))

        def khat(m):
            t1 = sqp.tile([128, T], BF16, tag="sqb")
            nc.vector.tensor_tensor(out=t1, in0=k_sb[:, m, :],
                                    in1=mrk[:, 0:1024], op=OP.subtract)
            t2 = sqp.tile([128, T], BF16, tag="sqb")
            nc.vector.tensor_tensor(out=t2, in0=t1,
                                    in1=mrk[:, 1024:2048], op=OP.mult)
            nc.scalar.activation(out=k_sb[:, m, :], in_=t2,
                                 func=AF.Identity, bias=kb8[:, m:m + 1],
                                 scale=kg8[:, m:m + 1])

        vgroup(0)
        # k broadcast, hidden under the v matmuls; mrk packs [mu0|mu1|rs0|rs1]
        for h in range(2):
            bckh = bckp.tile([128, 1024], F32, tag="bk")
            bc_half(muk[:, h * 512:(h + 1) * 512], rsk[:, h * 512:(h + 1) * 512], bckh)
            nc.vector.tensor_copy(out=mrk[:, h * 512:h * 512 + 512], in_=bckh[:, 0:512])
            nc.vector.tensor_copy(out=mrk[:, 1024 + h * 512:1024 + h * 512 + 512],
                                  in_=bckh[:, 512:1024])
        vgroup(1)
        khat(0)
        khat(1)
        vgroup(2)
        khat(2)
        khat(3)
        vgroup(3)
        for m in range(4, NCH):
            khat(m)

        bck_ctx.close()
        vps_ctx.close()
        kmm2 = None
        ks_ctx.close()
        qs_ctx.close()
        tmp_ctx.close()
        xz_ctx.close()

        # ---- attention: software-pipelined pairs ----
        att_ctx = ExitStack()
        pexpp = att_ctx.enter_context(pool(name="pexp", bufs=16))
        denp = att_ctx.enter_context(pool(name="den", bufs=1))
        rcbp = att_ctx.enter_context(pool(name="rcb", bufs=2))
        scp = att_ctx.enter_context(pool(name="sc", bufs=2, space="PSUM"))
        avp = att_ctx.enter_context(pool(name="av", bufs=4, space="PSUM"))

        p_tiles = {}
        av_tiles = {}

        def sc_group(m, i, av_mm=None):
            """scores+exp for pair m, chunk i; av_mm emits the interleaved
            av matmuls of the previous pair."""
            sc_ps = scp.tile([128, 1024], F32, tag="sc")
            nc.tensor.matmul(sc_ps[:, 0:512],
                             k_sb[0:64, m, i * 128:(i + 1) * 128],
                             q_sb[0:64, m, :], start=True, stop=True)
            nc.tensor.matmul(sc_ps[:, 512:1024],
                             k_sb[64:128, m, i * 128:(i + 1) * 128],
                             q_sb[64:128, m, :], start=True, stop=True)
            p_sb = pexpp.tile([128, 1024], BF16, tag="p")
            nc.scalar.activation(out=p_sb, in_=sc_ps, func=AF.Exp, scale=0.125)
            p_tiles[m].append(p_sb)
            if av_mm is not None:
                av_mm(i)

        def av_group_fn(mprev):
            p_list = p_tiles[mprev]
            h0, h1 = 2 * mprev, 2 * mprev + 1
            av0 = avp.tile([65, TQ], F32, tag="av")
            av1 = avp.tile([65, TQ], F32, tag="av")
            av_tiles[mprev] = (av0, av1)

            def av_mm(i):
                st, sp = i == 0, i == NCH - 1
                nc.tensor.matmul(av0, v_sb[:, i, h0 * 65:h0 * 65 + 65],
                                 p_list[i][:, 0:512], start=st, stop=sp)
                nc.tensor.matmul(av1, v_sb[:, i, h1 * 65:h1 * 65 + 65],
                                 p_list[i][:, 512:1024], start=st, stop=sp)
            return av_mm

        def av_drain(mprev):
            av0, av1 = av_tiles.pop(mprev)
            p_tiles.pop(mprev)
            dd = denp.tile([1, 2 * TQ], F32, tag="den")
            nc.vector.tensor_copy(out=dd[:, 0:TQ], in_=av0[64:65, :])
            nc.vector.tensor_copy(out=dd[:, TQ:2 * TQ], in_=av1[64:65, :])
            rt = denp.tile([1, 2 * TQ], F32, tag="rect")
            nc.vector.reciprocal_approx_fast(out=rt, in_=dd)
            rbb = rcbp.tile([64, 2 * TQ], F32, tag="rbb")
            nc.gpsimd.partition_broadcast(rbb, rt)
            nc.vector.tensor_tensor(out=outT_sb[0:64, mprev, :],
                                    in0=av0[0:64, :], in1=rbb[:, 0:TQ], op=OP.mult)
            nc.vector.tensor_tensor(out=outT_sb[64:128, mprev, :],
                                    in0=av1[0:64, :], in1=rbb[:, TQ:2 * TQ], op=OP.mult)

        p_tiles[0] = []
        for i in range(NCH):
            sc_group(0, i)
        for m in range(1, NCH):
            p_tiles[m] = []
            av_mm = av_group_fn(m - 1)
            for i in range(NCH):
                sc_group(m, i, av_mm)
            av_drain(m - 1)
        av_mm = av_group_fn(NCH - 1)
        for i in range(NCH):
            av_mm(i)
        av_drain(NCH - 1)

        att_ctx.close()

        # ================= phase D: proj =================
        youtp = ctx.enter_context(pool(name="yout", bufs=2))
        pjp = ctx.enter_context(pool(name="pj", bufs=2, space="PSUM"))
        for m in range(NCH):
            y_ps = pjp.tile([128, TQ], F32, tag="pj")
            for j in range(NCH):
                nc.tensor.matmul(y_ps, wp_v[:, j, m * 128:(m + 1) * 128], outT_sb[:, j, :],
                                 start=(j == 0), stop=(j == NCH - 1))
            y_sb = youtp.tile([128, TQ], F32, tag="y")
            nc.scalar.activation(out=y_sb, in_=y_ps, func=AF.Identity,
                                 bias=bp8[:, m:m + 1], scale=1.0)
            nc.sync.dma_start(out=yT_d[m * 128:(m + 1) * 128, :], in_=y_sb)

    nc.finalize()
    return nc


def _get_nc():
    if "nc" not in _CACHE:
        _CACHE["nc"] = _build()
    return _CACHE["nc"]


def _lay_w(w, gcols):
    """[C, C] -> [128, 8192] slab-contiguous: A[p, m, j, c'] = w[j*128+p, m*gcols+c']."""
    A = w.reshape(NCH, 128, C // gcols, gcols).transpose(1, 2, 0, 3)
    return np.ascontiguousarray(A.reshape(128, NCH * C))


def _prep_inputs(x, norm_g, norm_b, qkv_w, qkv_b, qln_g, qln_b, kln_g, kln_b, proj_w, proj_b):
    x = np.asarray(x, dtype=np.float32)
    norm_g = np.asarray(norm_g, dtype=np.float32)
    norm_b = np.asarray(norm_b, dtype=np.float32)
    qkv_w = np.asarray(qkv_w, dtype=np.float32)
    qkv_b = np.asarray(qkv_b, dtype=np.float32)
    proj_w = np.asarray(proj_w, dtype=np.float32)
    proj_b = np.asarray(proj_b, dtype=np.float32)

    wfold = norm_g[:, None] * qkv_w                    # [C, 3C]
    bfold = qkv_b + norm_b @ qkv_w                     # [3C]
    wq = np.ascontiguousarray(wfold[:, 0:C])
    wk = np.ascontiguousarray(wfold[:, C:2 * C])
    wv = np.ascontiguousarray(wfold[:, 2 * C:3 * C])
    bq, bk, bv = bfold[0:C].copy(), bfold[C:2 * C].copy(), bfold[2 * C:3 * C].copy()
    # v bias folds through attention (softmax rows sum to 1) into proj bias
    bp = proj_b + bv @ proj_w

    bf16 = ml_dtypes.bfloat16
    wp_lay = np.ascontiguousarray(
        proj_w.reshape(NCH, 128, C).transpose(1, 0, 2).reshape(128, NCH * C))
    common = dict(
        wq=_lay_w(wq, 128).astype(bf16), wk=_lay_w(wk, 128).astype(bf16),
        wv=_lay_w(wv, 256).astype(bf16), wp=wp_lay.astype(bf16),
        bq=bq, bk=bk, bp=bp,
        qg=np.asarray(qln_g, dtype=np.float32).copy(),
        qb=np.asarray(qln_b, dtype=np.float32).copy(),
        kg=np.asarray(kln_g, dtype=np.float32).copy(),
        kb=np.asarray(kln_b, dtype=np.float32).copy(),
    )
    in_maps = []
    for core in range(8):
        b, half = core // 2, core % 2
        xp = np.concatenate([x[b, TQ * half:], x[b, :TQ * half]], axis=0) if half else x[b]
        xT = np.ascontiguousarray(xp.T).astype(bf16)
        in_maps.append(dict(common, xT=xT))
    return in_maps


def kernel(**inputs) -> np.ndarray:
    in_maps = _prep_inputs(**inputs)
    nc = _get_nc()
    res = run_bass_kernel_spmd(nc, in_maps, core_ids=list(range(8)))
    out = np.empty((B, T, C), dtype=np.float32)
    for core in range(8):
        b, half = core // 2, core % 2
        out[b, TQ * half:TQ * half + TQ, :] = res.results[core]["yT"].T
    return out
